# revision 14
# baseline (speedup 1.0000x reference)
"""CIN (Compressed Interaction Network) forward kernel for Trainium2, 8 cores.

Reference computation (per batch b, per position d):
  h0 = x                                  # [F=64, D=64] fields
  for layer l in (1, 2):
    z[(i,j), d] = x[i, d] * h[j, d]       # outer product, K = F*F = 4096
    h[o, d] = relu(sum_c W_l[o, c] z[c, d] + b_l[o])
  pooled[2F] = sum_d concat(h1, h2)
  y = pooled @ Wfc.T + bfc

Distribution: pure data parallel — batch dim (1024) split 128/core across 8
NeuronCores; weights replicated; no collectives needed (forward only).

Per-core algorithm ("n" = flattened (batch, d) = 8192 columns):
  - x is host-pre-transposed to xt[i, n] fp16 and staged in DRAM.
  - DMA broadcasts xt rows into X_c tiles [128, n]: partition p of chunk c
    holds x-row (2c + p//64) — the i-side of the outer product. DMA is the
    only engine that can replicate across partitions at line rate.
  - DVE builds z_c = Hdup * X_c in fp16 (2x perf mode); Hdup[p, n] =
    h[p % 64, n] is the j-side (vertically duplicated h).
  - PE contracts: psum[o, n] += W_chunk^T @ z_c, two n-subtiles of 512 run
    concurrently on array column halves (M=64 each) -> full-array rate.
  - ACT applies bias+relu (fp32 psum -> fp16), assembles next layer's Hdup
    (one cross-partition copy via small SBUF->SBUF DMA).
  - Final FC is folded into PE (contract fields), sum-pool over d via DVE
    tensor_reduce on the [1, n] psum row.
"""

import numpy as np

import concourse.bass as bass
import concourse.bacc as bacc
import concourse.mybir as mybir
import concourse.tile as tile
from concourse.bass_utils import run_bass_kernel_spmd

F = 64          # fields
D = 64          # embedding dim
B = 1024        # full batch
NCORES = 8
B_LOC = B // NCORES          # 128 batches per core
NCH = (F * F) // 128         # 32 contraction chunks of 128
f16 = mybir.dt.float16
f32 = mybir.dt.float32


def build_cin_nc(b_loc=B_LOC, macro=1024, fused_xdma=True, tt_fuse=1):
    """Build the per-core Bass program. n = b_loc * D columns, processed in
    macro-tiles of `macro` columns (= 2 psum subtiles of macro/2).
    tt_fuse: number of contraction chunks whose z is built by one DVE
    tensor_tensor op (>1 re-reads the h-side via a stride-0 AP dim)."""
    n_total = b_loc * D
    assert n_total % macro == 0
    n_macro = n_total // macro
    half = macro // 2
    assert half <= 512  # psum bank limit for fp32

    nc = bacc.Bacc(None)

    xt = nc.dram_tensor("xt16", [F, n_total], f16, kind="ExternalInput")
    w1d = nc.dram_tensor("w1sb", [128, NCH * F], f16, kind="ExternalInput")
    w2d = nc.dram_tensor("w2sb", [128, NCH * F], f16, kind="ExternalInput")
    b1d = nc.dram_tensor("b1dup", [128, 1], f32, kind="ExternalInput")
    b2d = nc.dram_tensor("b2dup", [128, 1], f32, kind="ExternalInput")
    wfc1d = nc.dram_tensor("wfc1", [128, 1], f16, kind="ExternalInput")
    wfc2td = nc.dram_tensor("wfc2t", [128, 1], f16, kind="ExternalInput")
    wfc2bd = nc.dram_tensor("wfc2b", [128, 1], f16, kind="ExternalInput")
    bfcd = nc.dram_tensor("bfc", [1, 1], f32, kind="ExternalInput")
    yd = nc.dram_tensor("y", [1, b_loc], f32, kind="ExternalOutput")

    mult = mybir.AluOpType.mult
    Relu = mybir.ActivationFunctionType.Relu

    with tile.TileContext(nc) as tc:
        with (
            tc.tile_pool(name="const", bufs=1) as cpool,
            tc.tile_pool(name="xbig", bufs=2) as xpool,
            tc.tile_pool(name="z", bufs=6) as zpool,
            tc.tile_pool(name="h", bufs=2) as hpool,
            tc.tile_pool(name="psum", bufs=4, space="PSUM") as ppool,
            tc.tile_pool(name="psumfc", bufs=2, space="PSUM") as fcpool,
        ):
            # ---- constants / static tiles ----
            w1 = cpool.tile([128, NCH * F], f16)
            w2 = cpool.tile([128, NCH * F], f16)
            b1 = cpool.tile([128, 1], f32)
            b2 = cpool.tile([128, 1], f32)
            wfc1 = cpool.tile([128, 1], f16)
            wfc2t = cpool.tile([128, 1], f16)
            wfc2b = cpool.tile([128, 1], f16)
            bfc = cpool.tile([1, 1], f32)
            xdup = cpool.tile([128, n_total], f16)
            y_all = cpool.tile([1, b_loc], f32)

            nc.sync.dma_start(out=w1[:], in_=w1d[:])
            nc.sync.dma_start(out=w2[:], in_=w2d[:])
            nc.sync.dma_start(out=b1[:], in_=b1d[:])
            nc.sync.dma_start(out=b2[:], in_=b2d[:])
            nc.sync.dma_start(out=wfc1[:], in_=wfc1d[:])
            nc.sync.dma_start(out=wfc2t[:], in_=wfc2td[:])
            nc.sync.dma_start(out=wfc2b[:], in_=wfc2bd[:])
            nc.sync.dma_start(out=bfc[:], in_=bfcd[:])
            nc.sync.dma_start(out=xdup[0:64, :], in_=xt[:])
            nc.sync.dma_start(out=xdup[64:128, :], in_=xt[:])

            for m in range(n_macro):
                n0 = m * macro
                # ---- X broadcast tiles for this macro: [128, NCH*macro] ----
                X = xpool.tile([128, NCH * macro], f16, tag="X")
                if fused_xdma:
                    # partitions 0-63 <- even x-rows (i = 2c), 64-127 <- odd
                    src_rc = xt[:, n0 : n0 + macro].rearrange(
                        "(c r) n -> r c n", r=2
                    )  # [2, NCH, macro]
                    for r in range(2):
                        nc.sync.dma_start(
                            out=X[64 * r : 64 * (r + 1), :].rearrange(
                                "j (c n) -> j c n", n=macro
                            ),
                            in_=src_rc[r].unsqueeze(0).broadcast_to(
                                [64, NCH, macro]
                            ),
                        )
                else:
                    for c in range(NCH):
                        for r in range(2):
                            nc.sync.dma_start(
                                out=X[64 * r : 64 * (r + 1), c * macro : (c + 1) * macro],
                                in_=xt[2 * c + r : 2 * c + r + 1, n0 : n0 + macro]
                                .broadcast_to([64, macro]),
                            )

                # ---- layer 1: z = (x j-side) * (x i-side), contract -> psum
                # separate banks per col-group half: a bank never sees
                # concurrent accumulation groups / readers+writers
                ps1a = ppool.tile([128, half], f32, tag="ps")
                ps1b = ppool.tile([128, half], f32, tag="ps")
                for c0 in range(0, NCH, tt_fuse):
                    z = zpool.tile([128, tt_fuse * macro], f16, tag="z")
                    if tt_fuse == 1:
                        nc.vector.tensor_tensor(
                            z[:], xdup[:, n0 : n0 + macro],
                            X[:, c0 * macro : (c0 + 1) * macro], mult,
                        )
                    else:
                        nc.vector.tensor_tensor(
                            z[:].rearrange("p (f n) -> p f n", n=macro),
                            xdup[:, n0 : n0 + macro].unsqueeze(1)
                            .broadcast_to([128, tt_fuse, macro]),
                            X[:, c0 * macro : (c0 + tt_fuse) * macro]
                            .rearrange("p (f n) -> p f n", n=macro),
                            mult,
                        )
                    for cc in range(tt_fuse):
                        c = c0 + cc
                        wsl = w1[:, c * F : (c + 1) * F]
                        zoff = cc * macro
                        nc.tensor.matmul(
                            ps1a[0:64, :], wsl, z[:, zoff : zoff + half],
                            start=(c == 0), stop=(c == NCH - 1),
                        )
                        nc.tensor.matmul(
                            ps1b[64:128, :], wsl, z[:, zoff + half : zoff + macro],
                            start=(c == 0), stop=(c == NCH - 1),
                        )

                # ---- h1 = relu(psum1 + b1), duplicated to both halves ----
                # Hp[p, nn] = h1[p % 64, n0 + nn]
                Hp = hpool.tile([128, macro], f16, tag="Hp")
                nc.scalar.activation(
                    Hp[0:64, 0:half], ps1a[0:64, :], Relu, bias=b1[0:64, :]
                )
                nc.scalar.activation(
                    Hp[64:128, half:macro], ps1b[64:128, :], Relu, bias=b1[64:128, :]
                )
                # cross-partition duplicates (engines are lane-locked -> DMA)
                nc.sync.dma_start(out=Hp[64:128, 0:half], in_=Hp[0:64, 0:half])
                nc.sync.dma_start(out=Hp[0:64, half:macro], in_=Hp[64:128, half:macro])

                # ---- layer 2: z = h1 * (x i-side), contract -> psum ----
                ps2a = ppool.tile([128, half], f32, tag="ps")
                ps2b = ppool.tile([128, half], f32, tag="ps")
                for c0 in range(0, NCH, tt_fuse):
                    z = zpool.tile([128, tt_fuse * macro], f16, tag="z")
                    if tt_fuse == 1:
                        nc.vector.tensor_tensor(
                            z[:], Hp[:], X[:, c0 * macro : (c0 + 1) * macro], mult,
                        )
                    else:
                        nc.vector.tensor_tensor(
                            z[:].rearrange("p (f n) -> p f n", n=macro),
                            Hp[:].unsqueeze(1)
                            .broadcast_to([128, tt_fuse, macro]),
                            X[:, c0 * macro : (c0 + tt_fuse) * macro]
                            .rearrange("p (f n) -> p f n", n=macro),
                            mult,
                        )
                    for cc in range(tt_fuse):
                        c = c0 + cc
                        wsl = w2[:, c * F : (c + 1) * F]
                        zoff = cc * macro
                        nc.tensor.matmul(
                            ps2a[0:64, :], wsl, z[:, zoff : zoff + half],
                            start=(c == 0), stop=(c == NCH - 1),
                        )
                        nc.tensor.matmul(
                            ps2b[64:128, :], wsl, z[:, zoff + half : zoff + macro],
                            start=(c == 0), stop=(c == NCH - 1),
                        )

                # ---- h2 = relu(psum2 + b2) (no duplication needed) ----
                h2 = hpool.tile([128, half], f16, tag="h2")
                nc.scalar.activation(
                    h2[0:64, :], ps2a[0:64, :], Relu, bias=b2[0:64, :]
                )
                nc.scalar.activation(
                    h2[64:128, :], ps2b[64:128, :], Relu, bias=b2[64:128, :]
                )

                # ---- FC contraction over fields on PE ----
                # pfA covers n-subtile t (batches), pfB covers subtile t+1
                pfA = fcpool.tile([1, half], f32, tag="pf")
                pfB = fcpool.tile([1, half], f32, tag="pf")
                nc.tensor.matmul(
                    pfA[:], wfc1[0:64, :], Hp[0:64, 0:half], start=True, stop=False
                )
                nc.tensor.matmul(
                    pfA[:], wfc2t[:], h2[:, :], start=False, stop=True
                )
                nc.tensor.matmul(
                    pfB[:], wfc1[0:64, :], Hp[0:64, half:macro], start=True, stop=False
                )
                nc.tensor.matmul(
                    pfB[:], wfc2b[:], h2[:, :], start=False, stop=True
                )

                # ---- sum-pool over d: [1, (nb, D)] -> [1, nb] ----
                nbat = half // D
                c0 = m * (macro // D)
                nc.vector.tensor_reduce(
                    y_all[0:1, c0 : c0 + nbat],
                    pfA[0:1, :].rearrange("p (b d) -> p b d", d=D),
                    mybir.AxisListType.X,
                    mybir.AluOpType.add,
                )
                nc.vector.tensor_reduce(
                    y_all[0:1, c0 + nbat : c0 + 2 * nbat],
                    pfB[0:1, :].rearrange("p (b d) -> p b d", d=D),
                    mybir.AxisListType.X,
                    mybir.AluOpType.add,
                )

            # ---- final bias and store ----
            nc.vector.tensor_scalar_add(y_all[:], y_all[:], bfc[:])
            nc.sync.dma_start(out=yd[:], in_=y_all[:])

    return nc


def _prep_shared(W1, b1, W2, b2, Wfc, bfc):
    """Host-side weight relayout (replicated on every core)."""
    def lay_w(W):
        # w[p, c*F + o] = W[o, c*128 + p]
        wt = np.ascontiguousarray(W.T.astype(np.float16))      # [4096, 64]
        return np.ascontiguousarray(
            wt.reshape(NCH, 128, F).transpose(1, 0, 2).reshape(128, NCH * F)
        )

    shared = {
        "w1sb": lay_w(W1),
        "w2sb": lay_w(W2),
        "b1dup": np.concatenate([b1, b1]).reshape(128, 1).astype(np.float32),
        "b2dup": np.concatenate([b2, b2]).reshape(128, 1).astype(np.float32),
        "wfc1": np.concatenate([Wfc[0, :F], Wfc[0, :F]]).reshape(128, 1).astype(np.float16),
        "wfc2t": np.concatenate([Wfc[0, F:], np.zeros(F, np.float32)]).reshape(128, 1).astype(np.float16),
        "wfc2b": np.concatenate([np.zeros(F, np.float32), Wfc[0, F:]]).reshape(128, 1).astype(np.float16),
        "bfc": bfc.reshape(1, 1).astype(np.float32),
    }
    return shared


_NC_CACHE = {}


def _get_nc(key=(B_LOC, 1024, True, 1)):
    if key not in _NC_CACHE:
        nc = build_cin_nc(*key)
        nc.finalize()   # bacc legalization (wait splitting, reg alloc)
        _NC_CACHE[key] = nc
    return _NC_CACHE[key]


def run(x, W1, b1, W2, b2, Wfc, bfc, trace=False, macro=1024, fused_xdma=True,
        tt_fuse=1, **spmd_kwargs):
    x = np.asarray(x, dtype=np.float32)
    shared = _prep_shared(
        np.asarray(W1, np.float32), np.asarray(b1, np.float32),
        np.asarray(W2, np.float32), np.asarray(b2, np.float32),
        np.asarray(Wfc, np.float32), np.asarray(bfc, np.float32),
    )
    in_maps = []
    for c in range(NCORES):
        xc = x[c * B_LOC : (c + 1) * B_LOC]                    # [128, F, D]
        xtc = np.ascontiguousarray(
            xc.transpose(1, 0, 2).reshape(F, B_LOC * D).astype(np.float16)
        )
        in_maps.append({"xt16": xtc, **shared})
    nc = _get_nc((B_LOC, macro, fused_xdma, tt_fuse))
    res = run_bass_kernel_spmd(
        nc, in_maps, list(range(NCORES)), trace=trace, **spmd_kwargs
    )
    ys = [np.asarray(res.results[i]["y"]).reshape(B_LOC) for i in range(NCORES)]
    out = np.concatenate(ys).reshape(B, 1).astype(np.float32)
    return out, res


def kernel(x, W1, b1, W2, b2, Wfc, bfc):
    out, _ = run(x, W1, b1, W2, b2, Wfc, bfc, trace=False)
    return out


# revision 15
# speedup vs baseline: 1.1782x; 1.1782x over previous
"""CIN (Compressed Interaction Network) forward kernel for Trainium2, 8 cores.

Reference computation (per batch b, per position d):
  h0 = x                                  # [F=64, D=64] fields
  for layer l in (1, 2):
    z[(i,j), d] = x[i, d] * h[j, d]       # outer product, K = F*F = 4096
    h[o, d] = relu(sum_c W_l[o, c] z[c, d] + b_l[o])
  pooled[2F] = sum_d concat(h1, h2)
  y = pooled @ Wfc.T + bfc

Distribution: pure data parallel — batch dim (1024) split 128/core across 8
NeuronCores; weights replicated; no collectives needed (forward only).

Per-core algorithm ("n" = flattened (batch, d) = 8192 columns):
  - x is host-pre-transposed to xt[i, n] fp16 and staged in DRAM.
  - DMA broadcasts xt rows into X_c tiles [128, n]: partition p of chunk c
    holds x-row (2c + p//64) — the i-side of the outer product. DMA is the
    only engine that can replicate across partitions at line rate.
  - DVE builds z_c = Hdup * X_c in fp16 (2x perf mode); Hdup[p, n] =
    h[p % 64, n] is the j-side (vertically duplicated h).
  - PE contracts: psum[o, n] += W_chunk^T @ z_c, two n-subtiles of 512 run
    concurrently on array column halves (M=64 each) -> full-array rate.
  - ACT applies bias+relu (fp32 psum -> fp16), assembles next layer's Hdup
    (one cross-partition copy via small SBUF->SBUF DMA).
  - Final FC is folded into PE (contract fields), sum-pool over d via DVE
    tensor_reduce on the [1, n] psum row.
"""

import numpy as np

import concourse.bass as bass
import concourse.bacc as bacc
import concourse.mybir as mybir
import concourse.tile as tile
from concourse.bass_utils import run_bass_kernel_spmd

F = 64          # fields
D = 64          # embedding dim
B = 1024        # full batch
NCORES = 8
B_LOC = B // NCORES          # 128 batches per core
NCH = (F * F) // 128         # 32 contraction chunks of 128
f16 = mybir.dt.float16
f32 = mybir.dt.float32


def build_cin_nc(b_loc=B_LOC, macro=1024, fused_xdma=True, tt_fuse=1):
    """Build the per-core Bass program. n = b_loc * D columns, processed in
    macro-tiles of `macro` columns (= 2 psum subtiles of macro/2).
    tt_fuse: number of contraction chunks whose z is built by one DVE
    tensor_tensor op (>1 re-reads the h-side via a stride-0 AP dim)."""
    n_total = b_loc * D
    assert n_total % macro == 0
    n_macro = n_total // macro
    half = macro // 2
    assert half <= 512  # psum bank limit for fp32

    nc = bacc.Bacc(None)

    xt = nc.dram_tensor("xt16", [F, n_total], f16, kind="ExternalInput")
    w1d = nc.dram_tensor("w1sb", [128, NCH * F], f16, kind="ExternalInput")
    w2d = nc.dram_tensor("w2sb", [128, NCH * F], f16, kind="ExternalInput")
    b1d = nc.dram_tensor("b1dup", [128, 1], f32, kind="ExternalInput")
    b2d = nc.dram_tensor("b2dup", [128, 1], f32, kind="ExternalInput")
    wfc1d = nc.dram_tensor("wfc1", [128, 1], f16, kind="ExternalInput")
    wfc2td = nc.dram_tensor("wfc2t", [128, 1], f16, kind="ExternalInput")
    wfc2bd = nc.dram_tensor("wfc2b", [128, 1], f16, kind="ExternalInput")
    bfcd = nc.dram_tensor("bfc", [1, 1], f32, kind="ExternalInput")
    yd = nc.dram_tensor("y", [1, b_loc], f32, kind="ExternalOutput")

    mult = mybir.AluOpType.mult
    Relu = mybir.ActivationFunctionType.Relu

    with tile.TileContext(nc) as tc:
        with (
            tc.tile_pool(name="const", bufs=1) as cpool,
            tc.tile_pool(name="xbig", bufs=2) as xpool,
            tc.tile_pool(name="z", bufs=6) as zpool,
            tc.tile_pool(name="h", bufs=2) as hpool,
            tc.tile_pool(name="psum", bufs=4, space="PSUM") as ppool,
            tc.tile_pool(name="psumfc", bufs=2, space="PSUM") as fcpool,
        ):
            # ---- constants / static tiles ----
            w1 = cpool.tile([128, NCH * F], f16)
            w2 = cpool.tile([128, NCH * F], f16)
            b1 = cpool.tile([128, 1], f32)
            b2 = cpool.tile([128, 1], f32)
            wfc1 = cpool.tile([128, 1], f16)
            wfc2t = cpool.tile([128, 1], f16)
            wfc2b = cpool.tile([128, 1], f16)
            bfc = cpool.tile([1, 1], f32)
            xdup = cpool.tile([128, n_total], f16)
            y_all = cpool.tile([1, b_loc], f32)

            nc.sync.dma_start(out=w1[:], in_=w1d[:])
            nc.sync.dma_start(out=w2[:], in_=w2d[:])
            nc.sync.dma_start(out=b1[:], in_=b1d[:])
            nc.sync.dma_start(out=b2[:], in_=b2d[:])
            nc.sync.dma_start(out=wfc1[:], in_=wfc1d[:])
            nc.sync.dma_start(out=wfc2t[:], in_=wfc2td[:])
            nc.sync.dma_start(out=wfc2b[:], in_=wfc2bd[:])
            nc.sync.dma_start(out=bfc[:], in_=bfcd[:])
            nc.sync.dma_start(out=xdup[0:64, :], in_=xt[:])
            nc.sync.dma_start(out=xdup[64:128, :], in_=xt[:])

            for m in range(n_macro):
                n0 = m * macro
                # ---- X broadcast tiles for this macro: [128, NCH*macro] ----
                X = xpool.tile([128, NCH * macro], f16, tag="X")
                if fused_xdma:
                    # partitions 0-63 <- even x-rows (i = 2c), 64-127 <- odd
                    src_rc = xt[:, n0 : n0 + macro].rearrange(
                        "(c r) n -> r c n", r=2
                    )  # [2, NCH, macro]
                    for r in range(2):
                        nc.sync.dma_start(
                            out=X[64 * r : 64 * (r + 1), :].rearrange(
                                "j (c n) -> j c n", n=macro
                            ),
                            in_=src_rc[r].unsqueeze(0).broadcast_to(
                                [64, NCH, macro]
                            ),
                        )
                else:
                    for c in range(NCH):
                        for r in range(2):
                            nc.sync.dma_start(
                                out=X[64 * r : 64 * (r + 1), c * macro : (c + 1) * macro],
                                in_=xt[2 * c + r : 2 * c + r + 1, n0 : n0 + macro]
                                .broadcast_to([64, macro]),
                            )

                # ---- layer 1: z = (x j-side) * (x i-side), contract -> psum
                # separate banks per col-group half: a bank never sees
                # concurrent accumulation groups / readers+writers
                ps1a = ppool.tile([128, half], f32, tag="ps")
                ps1b = ppool.tile([128, half], f32, tag="ps")
                for c0 in range(0, NCH, tt_fuse):
                    z = zpool.tile([128, tt_fuse * macro], f16, tag="z")
                    if tt_fuse == 1:
                        nc.vector.tensor_tensor(
                            z[:], xdup[:, n0 : n0 + macro],
                            X[:, c0 * macro : (c0 + 1) * macro], mult,
                        )
                    else:
                        nc.vector.tensor_tensor(
                            z[:].rearrange("p (f n) -> p f n", n=macro),
                            xdup[:, n0 : n0 + macro].unsqueeze(1)
                            .broadcast_to([128, tt_fuse, macro]),
                            X[:, c0 * macro : (c0 + tt_fuse) * macro]
                            .rearrange("p (f n) -> p f n", n=macro),
                            mult,
                        )
                    for cc in range(tt_fuse):
                        c = c0 + cc
                        wsl = w1[:, c * F : (c + 1) * F]
                        zoff = cc * macro
                        nc.tensor.matmul(
                            ps1a[0:64, :], wsl, z[:, zoff : zoff + half],
                            start=(c == 0), stop=(c == NCH - 1),
                        )
                        nc.tensor.matmul(
                            ps1b[64:128, :], wsl, z[:, zoff + half : zoff + macro],
                            start=(c == 0), stop=(c == NCH - 1),
                        )

                # ---- h1 = relu(psum1 + b1), duplicated to both halves ----
                # Hp[p, nn] = h1[p % 64, n0 + nn]
                Hp = hpool.tile([128, macro], f16, tag="Hp")
                nc.scalar.activation(
                    Hp[0:64, 0:half], ps1a[0:64, :], Relu, bias=b1[0:64, :]
                )
                nc.scalar.activation(
                    Hp[64:128, half:macro], ps1b[64:128, :], Relu, bias=b1[64:128, :]
                )
                # cross-partition duplicates (engines are lane-locked -> DMA)
                nc.scalar.dma_start(out=Hp[64:128, 0:half], in_=Hp[0:64, 0:half])
                nc.scalar.dma_start(out=Hp[0:64, half:macro], in_=Hp[64:128, half:macro])

                # ---- layer 2: z = h1 * (x i-side), contract -> psum ----
                ps2a = ppool.tile([128, half], f32, tag="ps")
                ps2b = ppool.tile([128, half], f32, tag="ps")
                for c0 in range(0, NCH, tt_fuse):
                    z = zpool.tile([128, tt_fuse * macro], f16, tag="z")
                    if tt_fuse == 1:
                        nc.vector.tensor_tensor(
                            z[:], Hp[:], X[:, c0 * macro : (c0 + 1) * macro], mult,
                        )
                    else:
                        nc.vector.tensor_tensor(
                            z[:].rearrange("p (f n) -> p f n", n=macro),
                            Hp[:].unsqueeze(1)
                            .broadcast_to([128, tt_fuse, macro]),
                            X[:, c0 * macro : (c0 + tt_fuse) * macro]
                            .rearrange("p (f n) -> p f n", n=macro),
                            mult,
                        )
                    for cc in range(tt_fuse):
                        c = c0 + cc
                        wsl = w2[:, c * F : (c + 1) * F]
                        zoff = cc * macro
                        nc.tensor.matmul(
                            ps2a[0:64, :], wsl, z[:, zoff : zoff + half],
                            start=(c == 0), stop=(c == NCH - 1),
                        )
                        nc.tensor.matmul(
                            ps2b[64:128, :], wsl, z[:, zoff + half : zoff + macro],
                            start=(c == 0), stop=(c == NCH - 1),
                        )

                # ---- h2 = relu(psum2 + b2) (no duplication needed) ----
                h2 = hpool.tile([128, half], f16, tag="h2")
                nc.scalar.activation(
                    h2[0:64, :], ps2a[0:64, :], Relu, bias=b2[0:64, :]
                )
                nc.scalar.activation(
                    h2[64:128, :], ps2b[64:128, :], Relu, bias=b2[64:128, :]
                )

                # ---- FC contraction over fields on PE ----
                # pfA covers n-subtile t (batches), pfB covers subtile t+1
                pfA = fcpool.tile([1, half], f32, tag="pf")
                pfB = fcpool.tile([1, half], f32, tag="pf")
                nc.tensor.matmul(
                    pfA[:], wfc1[0:64, :], Hp[0:64, 0:half], start=True, stop=False
                )
                nc.tensor.matmul(
                    pfA[:], wfc2t[:], h2[:, :], start=False, stop=True
                )
                nc.tensor.matmul(
                    pfB[:], wfc1[0:64, :], Hp[0:64, half:macro], start=True, stop=False
                )
                nc.tensor.matmul(
                    pfB[:], wfc2b[:], h2[:, :], start=False, stop=True
                )

                # ---- sum-pool over d: [1, (nb, D)] -> [1, nb] ----
                nbat = half // D
                c0 = m * (macro // D)
                nc.vector.tensor_reduce(
                    y_all[0:1, c0 : c0 + nbat],
                    pfA[0:1, :].rearrange("p (b d) -> p b d", d=D),
                    mybir.AxisListType.X,
                    mybir.AluOpType.add,
                )
                nc.vector.tensor_reduce(
                    y_all[0:1, c0 + nbat : c0 + 2 * nbat],
                    pfB[0:1, :].rearrange("p (b d) -> p b d", d=D),
                    mybir.AxisListType.X,
                    mybir.AluOpType.add,
                )

            # ---- final bias and store ----
            nc.vector.tensor_scalar_add(y_all[:], y_all[:], bfc[:])
            nc.scalar.dma_start(out=yd[:], in_=y_all[:])

    return nc


def _prep_shared(W1, b1, W2, b2, Wfc, bfc):
    """Host-side weight relayout (replicated on every core)."""
    def lay_w(W):
        # w[p, c*F + o] = W[o, c*128 + p]
        wt = np.ascontiguousarray(W.T.astype(np.float16))      # [4096, 64]
        return np.ascontiguousarray(
            wt.reshape(NCH, 128, F).transpose(1, 0, 2).reshape(128, NCH * F)
        )

    shared = {
        "w1sb": lay_w(W1),
        "w2sb": lay_w(W2),
        "b1dup": np.concatenate([b1, b1]).reshape(128, 1).astype(np.float32),
        "b2dup": np.concatenate([b2, b2]).reshape(128, 1).astype(np.float32),
        "wfc1": np.concatenate([Wfc[0, :F], Wfc[0, :F]]).reshape(128, 1).astype(np.float16),
        "wfc2t": np.concatenate([Wfc[0, F:], np.zeros(F, np.float32)]).reshape(128, 1).astype(np.float16),
        "wfc2b": np.concatenate([np.zeros(F, np.float32), Wfc[0, F:]]).reshape(128, 1).astype(np.float16),
        "bfc": bfc.reshape(1, 1).astype(np.float32),
    }
    return shared


_NC_CACHE = {}


def _get_nc(key=(B_LOC, 1024, True, 1)):
    if key not in _NC_CACHE:
        nc = build_cin_nc(*key)
        nc.finalize()   # bacc legalization (wait splitting, reg alloc)
        _NC_CACHE[key] = nc
    return _NC_CACHE[key]


def run(x, W1, b1, W2, b2, Wfc, bfc, trace=False, macro=1024, fused_xdma=True,
        tt_fuse=1, **spmd_kwargs):
    x = np.asarray(x, dtype=np.float32)
    shared = _prep_shared(
        np.asarray(W1, np.float32), np.asarray(b1, np.float32),
        np.asarray(W2, np.float32), np.asarray(b2, np.float32),
        np.asarray(Wfc, np.float32), np.asarray(bfc, np.float32),
    )
    in_maps = []
    for c in range(NCORES):
        xc = x[c * B_LOC : (c + 1) * B_LOC]                    # [128, F, D]
        xtc = np.ascontiguousarray(
            xc.transpose(1, 0, 2).reshape(F, B_LOC * D).astype(np.float16)
        )
        in_maps.append({"xt16": xtc, **shared})
    nc = _get_nc((B_LOC, macro, fused_xdma, tt_fuse))
    res = run_bass_kernel_spmd(
        nc, in_maps, list(range(NCORES)), trace=trace, **spmd_kwargs
    )
    ys = [np.asarray(res.results[i]["y"]).reshape(B_LOC) for i in range(NCORES)]
    out = np.concatenate(ys).reshape(B, 1).astype(np.float32)
    return out, res


def kernel(x, W1, b1, W2, b2, Wfc, bfc):
    out, _ = run(x, W1, b1, W2, b2, Wfc, bfc, trace=False)
    return out


# revision 16
# speedup vs baseline: 1.2518x; 1.0625x over previous
"""CIN (Compressed Interaction Network) forward kernel for Trainium2, 8 cores.

Reference computation (per batch b, per position d):
  h0 = x                                  # [F=64, D=64] fields
  for layer l in (1, 2):
    z[(i,j), d] = x[i, d] * h[j, d]       # outer product, K = F*F = 4096
    h[o, d] = relu(sum_c W_l[o, c] z[c, d] + b_l[o])
  pooled[2F] = sum_d concat(h1, h2)
  y = pooled @ Wfc.T + bfc

Distribution: pure data parallel — batch dim (1024) split 128/core across 8
NeuronCores; weights replicated; no collectives (forward only).

Per-core algorithm ("n" = flattened (batch, d) = 8192 columns, processed in
macro-tiles of 1024 columns):
  - The i-side operand X_c[p, n] = x[2c + p//64, n] (row broadcast across 64
    partitions) is pre-replicated BY THE HOST into a per-macro-blocked DRAM
    tensor, so on-device it's one flat contiguous 8MB DMA per macro at HBM
    line rate (128 descriptors), instead of a scattered broadcast pattern.
  - DVE builds z = Hdup * X in fp16 (2x perf mode), two chunks per op
    (tt_fuse) via a stride-0 middle dim on the h-side AP.
  - PE contracts psum[o, :] += W_chunk^T @ z, the two 512-col n-subtiles
    running concurrently on array column halves (M=64 each, auto
    tile_position) -> full-array rate. Separate PSUM banks per half.
  - ACT applies bias+relu (fp32 psum -> fp16 Hdup); one cross-partition
    duplicate via small SBUF->SBUF DMAs on the scalar HWDGE ring (separate
    from the X ring to avoid head-of-line blocking).
  - Final FC is folded into PE (contract fields, K=128 with zero-padded
    wfc halves), sum-pool over d via DVE tensor_reduce on [1, n] psum.
  - Software pipelined depth 2: L1 of macro m+1 is emitted before L2 of
    macro m, so the DVE never idles across the layer boundary.
"""

import numpy as np

import concourse.bacc as bacc
import concourse.mybir as mybir
import concourse.tile as tile
from concourse.bass_utils import run_bass_kernel_spmd

F = 64          # fields
D = 64          # embedding dim
B = 1024        # full batch
NCORES = 8
B_LOC = B // NCORES          # 128 batches per core
NCH = (F * F) // 128         # 32 contraction chunks of 128
f16 = mybir.dt.float16
f32 = mybir.dt.float32


def build_cin_nc(b_loc=B_LOC, macro=1024, tt_fuse=2):
    n_total = b_loc * D
    assert n_total % macro == 0
    n_macro = n_total // macro
    half = macro // 2
    assert half <= 512  # psum bank limit for fp32
    assert NCH % tt_fuse == 0

    nc = bacc.Bacc(None)

    xt = nc.dram_tensor("xt16", [F, n_total], f16, kind="ExternalInput")
    xrep = nc.dram_tensor(
        "xrep", [n_macro * 128, NCH * macro], f16, kind="ExternalInput"
    )
    w1d = nc.dram_tensor("w1sb", [128, NCH * F], f16, kind="ExternalInput")
    w2d = nc.dram_tensor("w2sb", [128, NCH * F], f16, kind="ExternalInput")
    b1d = nc.dram_tensor("b1dup", [128, 1], f32, kind="ExternalInput")
    b2d = nc.dram_tensor("b2dup", [128, 1], f32, kind="ExternalInput")
    wfc1d = nc.dram_tensor("wfc1", [128, 1], f16, kind="ExternalInput")
    wfc2td = nc.dram_tensor("wfc2t", [128, 1], f16, kind="ExternalInput")
    wfc2bd = nc.dram_tensor("wfc2b", [128, 1], f16, kind="ExternalInput")
    bfcd = nc.dram_tensor("bfc", [1, 1], f32, kind="ExternalInput")
    yd = nc.dram_tensor("y", [1, b_loc], f32, kind="ExternalOutput")

    mult = mybir.AluOpType.mult
    Relu = mybir.ActivationFunctionType.Relu

    with tile.TileContext(nc) as tc:
        with (
            tc.tile_pool(name="const", bufs=1) as cpool,
            tc.tile_pool(name="xbig", bufs=2) as xpool,
            tc.tile_pool(name="xd", bufs=3) as xdpool,
            tc.tile_pool(name="z", bufs=6) as zpool,
            tc.tile_pool(name="h", bufs=2) as hpool,
            tc.tile_pool(name="psum", bufs=4, space="PSUM") as ppool,
            tc.tile_pool(name="psumfc", bufs=2, space="PSUM") as fcpool,
        ):
            # ---- constants ----
            w1 = cpool.tile([128, NCH * F], f16)
            w2 = cpool.tile([128, NCH * F], f16)
            b1 = cpool.tile([128, 1], f32)
            b2 = cpool.tile([128, 1], f32)
            wfc1 = cpool.tile([128, 1], f16)
            wfc2t = cpool.tile([128, 1], f16)
            wfc2b = cpool.tile([128, 1], f16)
            bfc = cpool.tile([1, 1], f32)
            y_all = cpool.tile([1, b_loc], f32)

            nc.scalar.dma_start(out=w1[:], in_=w1d[:])
            nc.scalar.dma_start(out=w2[:], in_=w2d[:])
            nc.scalar.dma_start(out=b1[:], in_=b1d[:])
            nc.scalar.dma_start(out=b2[:], in_=b2d[:])
            nc.scalar.dma_start(out=wfc1[:], in_=wfc1d[:])
            nc.scalar.dma_start(out=wfc2t[:], in_=wfc2td[:])
            nc.scalar.dma_start(out=wfc2b[:], in_=wfc2bd[:])
            nc.scalar.dma_start(out=bfc[:], in_=bfcd[:])

            Xs = {}       # macro -> X tile
            xds = {}      # macro -> xdup tile
            Hps = {}      # macro -> Hp tile

            def emit_load(m):
                n0 = m * macro
                X = xpool.tile([128, NCH * macro], f16, tag="X")
                # one flat contiguous 8MB copy (host pre-replicated layout)
                nc.sync.dma_start(out=X[:], in_=xrep[m * 128 : (m + 1) * 128, :])
                xd = xdpool.tile([128, macro], f16, tag="xd")
                nc.scalar.dma_start(out=xd[0:64, :], in_=xt[:, n0 : n0 + macro])
                nc.scalar.dma_start(out=xd[64:128, :], in_=xt[:, n0 : n0 + macro])
                Xs[m], xds[m] = X, xd

            def emit_layer(m, w, bvec, hdup_in, X):
                """One CIN layer: z build + contraction + relu epilogue.
                Returns (ha, hb) = psum halves after matmul (pre-activation)."""
                psa = ppool.tile([128, half], f32, tag="ps")
                psb = ppool.tile([128, half], f32, tag="ps")
                for c0 in range(0, NCH, tt_fuse):
                    z = zpool.tile([128, tt_fuse * macro], f16, tag="z")
                    nc.vector.tensor_tensor(
                        z[:].rearrange("p (f n) -> p f n", n=macro),
                        hdup_in.unsqueeze(1).broadcast_to([128, tt_fuse, macro]),
                        X[:, c0 * macro : (c0 + tt_fuse) * macro]
                        .rearrange("p (f n) -> p f n", n=macro),
                        mult,
                    )
                    for cc in range(tt_fuse):
                        c = c0 + cc
                        wsl = w[:, c * F : (c + 1) * F]
                        zoff = cc * macro
                        nc.tensor.matmul(
                            psa[0:64, :], wsl, z[:, zoff : zoff + half],
                            start=(c == 0), stop=(c == NCH - 1),
                        )
                        nc.tensor.matmul(
                            psb[64:128, :], wsl, z[:, zoff + half : zoff + macro],
                            start=(c == 0), stop=(c == NCH - 1),
                        )
                return psa, psb

            def emit_l1(m):
                psa, psb = emit_layer(m, w1, b1, xds[m][:, :], Xs[m])
                Hp = hpool.tile([128, macro], f16, tag="Hp")
                nc.scalar.activation(
                    Hp[0:64, 0:half], psa[0:64, :], Relu, bias=b1[0:64, :]
                )
                nc.scalar.activation(
                    Hp[64:128, half:macro], psb[64:128, :], Relu, bias=b1[64:128, :]
                )
                # cross-partition duplicates (engines are lane-locked -> DMA)
                nc.scalar.dma_start(out=Hp[64:128, 0:half], in_=Hp[0:64, 0:half])
                nc.scalar.dma_start(
                    out=Hp[0:64, half:macro], in_=Hp[64:128, half:macro]
                )
                Hps[m] = Hp

            def emit_l2(m):
                Hp, X = Hps[m], Xs[m]
                psa, psb = emit_layer(m, w2, b2, Hp[:, :], X)
                h2 = hpool.tile([128, half], f16, tag="h2")
                nc.scalar.activation(
                    h2[0:64, :], psa[0:64, :], Relu, bias=b2[0:64, :]
                )
                nc.scalar.activation(
                    h2[64:128, :], psb[64:128, :], Relu, bias=b2[64:128, :]
                )
                # FC over fields on PE; pfA = n-subtile t, pfB = subtile t+1
                pfA = fcpool.tile([1, half], f32, tag="pf")
                pfB = fcpool.tile([1, half], f32, tag="pf")
                nc.tensor.matmul(
                    pfA[:], wfc1[0:64, :], Hp[0:64, 0:half], start=True, stop=False
                )
                nc.tensor.matmul(pfA[:], wfc2t[:], h2[:, :], start=False, stop=True)
                nc.tensor.matmul(
                    pfB[:], wfc1[0:64, :], Hp[0:64, half:macro],
                    start=True, stop=False,
                )
                nc.tensor.matmul(pfB[:], wfc2b[:], h2[:, :], start=False, stop=True)
                # sum-pool over d
                nbat = half // D
                c0 = m * (macro // D)
                nc.vector.tensor_reduce(
                    y_all[0:1, c0 : c0 + nbat],
                    pfA[0:1, :].rearrange("p (b d) -> p b d", d=D),
                    mybir.AxisListType.X, mybir.AluOpType.add,
                )
                nc.vector.tensor_reduce(
                    y_all[0:1, c0 + nbat : c0 + 2 * nbat],
                    pfB[0:1, :].rearrange("p (b d) -> p b d", d=D),
                    mybir.AxisListType.X, mybir.AluOpType.add,
                )
                del Hps[m], Xs[m], xds[m]

            # ---- depth-2 software pipeline over macro tiles ----
            emit_load(0)
            emit_l1(0)
            for m in range(n_macro):
                if m + 1 < n_macro:
                    emit_load(m + 1)
                    emit_l1(m + 1)
                emit_l2(m)

            nc.vector.tensor_scalar_add(y_all[:], y_all[:], bfc[:])
            nc.scalar.dma_start(out=yd[:], in_=y_all[:])

    return nc


def _prep_shared(W1, b1, W2, b2, Wfc, bfc):
    """Host-side weight relayout (replicated on every core)."""
    def lay_w(W):
        # w[p, c*F + o] = W[o, c*128 + p]
        wt = np.ascontiguousarray(W.T.astype(np.float16))      # [4096, 64]
        return np.ascontiguousarray(
            wt.reshape(NCH, 128, F).transpose(1, 0, 2).reshape(128, NCH * F)
        )

    return {
        "w1sb": lay_w(W1),
        "w2sb": lay_w(W2),
        "b1dup": np.concatenate([b1, b1]).reshape(128, 1).astype(np.float32),
        "b2dup": np.concatenate([b2, b2]).reshape(128, 1).astype(np.float32),
        "wfc1": np.concatenate([Wfc[0, :F], Wfc[0, :F]]).reshape(128, 1).astype(np.float16),
        "wfc2t": np.concatenate([Wfc[0, F:], np.zeros(F, np.float32)]).reshape(128, 1).astype(np.float16),
        "wfc2b": np.concatenate([np.zeros(F, np.float32), Wfc[0, F:]]).reshape(128, 1).astype(np.float16),
        "bfc": bfc.reshape(1, 1).astype(np.float32),
    }


def _prep_x(xt, b_loc, macro=1024):
    """Build the per-macro-blocked replicated X layout.
    xrep[m*128 + p, c*macro + nn] = xt[2c + p//64, m*macro + nn]"""
    n_total = b_loc * D
    n_macro = n_total // macro
    xm = xt.reshape(F, n_macro, macro)                   # [row, m, nn]
    # rows for (r, c): 2c + r ; partition p = r*64 + j (j broadcast)
    sel = xm.reshape(NCH, 2, n_macro, macro)             # [c, r, m, nn]
    rep = np.broadcast_to(
        sel.transpose(2, 1, 0, 3)[:, :, None, :, :],     # [m, r, 1, c, nn]
        (n_macro, 2, 64, NCH, macro),
    )
    return np.ascontiguousarray(rep).reshape(n_macro * 128, NCH * macro)


_NC_CACHE = {}


def _get_nc(key=(B_LOC, 1024, 2)):
    if key not in _NC_CACHE:
        nc = build_cin_nc(*key)
        nc.finalize()   # bacc legalization (wait splitting, reg alloc)
        _NC_CACHE[key] = nc
    return _NC_CACHE[key]


def run(x, W1, b1, W2, b2, Wfc, bfc, trace=False, macro=1024, tt_fuse=2,
        **spmd_kwargs):
    x = np.asarray(x, dtype=np.float32)
    shared = _prep_shared(
        np.asarray(W1, np.float32), np.asarray(b1, np.float32),
        np.asarray(W2, np.float32), np.asarray(b2, np.float32),
        np.asarray(Wfc, np.float32), np.asarray(bfc, np.float32),
    )
    in_maps = []
    for c in range(NCORES):
        xc = x[c * B_LOC : (c + 1) * B_LOC]                    # [128, F, D]
        xtc = np.ascontiguousarray(
            xc.transpose(1, 0, 2).reshape(F, B_LOC * D).astype(np.float16)
        )
        in_maps.append(
            {"xt16": xtc, "xrep": _prep_x(xtc, B_LOC, macro), **shared}
        )
    nc = _get_nc((B_LOC, macro, tt_fuse))
    res = run_bass_kernel_spmd(
        nc, in_maps, list(range(NCORES)), trace=trace, **spmd_kwargs
    )
    ys = [np.asarray(res.results[i]["y"]).reshape(B_LOC) for i in range(NCORES)]
    out = np.concatenate(ys).reshape(B, 1).astype(np.float32)
    return out, res


def kernel(x, W1, b1, W2, b2, Wfc, bfc):
    out, _ = run(x, W1, b1, W2, b2, Wfc, bfc, trace=False)
    return out


# revision 17
# speedup vs baseline: 1.4523x; 1.1601x over previous
"""CIN (Compressed Interaction Network) forward kernel for Trainium2, 8 cores.

Reference computation (per batch b, per position d):
  h0 = x                                  # [F=64, D=64] fields
  for layer l in (1, 2):
    z[(i,j), d] = x[i, d] * h[j, d]       # outer product, K = F*F = 4096
    h[o, d] = relu(sum_c W_l[o, c] z[c, d] + b_l[o])
  pooled[2F] = sum_d concat(h1, h2)
  y = pooled @ Wfc.T + bfc

Distribution: pure data parallel — batch dim (1024) split 128/core across 8
NeuronCores; weights replicated; no collectives (forward only).

Per-core algorithm ("n" = flattened (batch, d) = 8192 columns, processed in
macro-tiles of 1024 columns):
  - The i-side operand X_c[p, n] = x[2c + p//64, n] (row broadcast across 64
    partitions) is pre-replicated BY THE HOST into a per-macro-blocked DRAM
    tensor, so on-device it's one flat contiguous 8MB DMA per macro at HBM
    line rate (128 descriptors), instead of a scattered broadcast pattern.
  - DVE builds z = Hdup * X in fp16 (2x perf mode), two chunks per op
    (tt_fuse) via a stride-0 middle dim on the h-side AP.
  - PE contracts psum[o, :] += W_chunk^T @ z, the two 512-col n-subtiles
    running concurrently on array column halves (M=64 each, auto
    tile_position) -> full-array rate. Separate PSUM banks per half.
  - ACT applies bias+relu (fp32 psum -> fp16 Hdup); one cross-partition
    duplicate via small SBUF->SBUF DMAs on the scalar HWDGE ring (separate
    from the X ring to avoid head-of-line blocking).
  - Final FC is folded into PE (contract fields, K=128 with zero-padded
    wfc halves), sum-pool over d via DVE tensor_reduce on [1, n] psum.
  - Software pipelined depth 2: L1 of macro m+1 is emitted before L2 of
    macro m, so the DVE never idles across the layer boundary.
"""

import numpy as np

import concourse.bacc as bacc
import concourse.mybir as mybir
import concourse.tile as tile
from concourse.bass_utils import run_bass_kernel_spmd

F = 64          # fields
D = 64          # embedding dim
B = 1024        # full batch
NCORES = 8
B_LOC = B // NCORES          # 128 batches per core
NCH = (F * F) // 128         # 32 contraction chunks of 128
f16 = mybir.dt.float16
f32 = mybir.dt.float32


def build_cin_nc(b_loc=B_LOC, macro=1024, tt_fuse=2):
    n_total = b_loc * D
    assert n_total % macro == 0
    n_macro = n_total // macro
    half = macro // 2
    assert half <= 512  # psum bank limit for fp32
    assert NCH % tt_fuse == 0

    nc = bacc.Bacc(None)

    xt = nc.dram_tensor("xt16", [F, n_total], f16, kind="ExternalInput")
    xrep = nc.dram_tensor(
        "xrep", [n_macro * 128, NCH * macro], f16, kind="ExternalInput"
    )
    w1d = nc.dram_tensor("w1sb", [128, NCH * F], f16, kind="ExternalInput")
    w2d = nc.dram_tensor("w2sb", [128, NCH * F], f16, kind="ExternalInput")
    b1d = nc.dram_tensor("b1dup", [128, 1], f32, kind="ExternalInput")
    b2d = nc.dram_tensor("b2dup", [128, 1], f32, kind="ExternalInput")
    wfc1d = nc.dram_tensor("wfc1", [128, 1], f16, kind="ExternalInput")
    wfc2td = nc.dram_tensor("wfc2t", [128, 1], f16, kind="ExternalInput")
    wfc2bd = nc.dram_tensor("wfc2b", [128, 1], f16, kind="ExternalInput")
    bfcd = nc.dram_tensor("bfc", [1, 1], f32, kind="ExternalInput")
    yd = nc.dram_tensor("y", [1, b_loc], f32, kind="ExternalOutput")

    mult = mybir.AluOpType.mult
    Relu = mybir.ActivationFunctionType.Relu

    with tile.TileContext(nc) as tc:
        with (
            tc.tile_pool(name="const", bufs=1) as cpool,
            tc.tile_pool(name="xbig", bufs=2) as xpool,
            tc.tile_pool(name="xd", bufs=3) as xdpool,
            tc.tile_pool(name="z", bufs=6) as zpool,
            tc.tile_pool(name="h", bufs=2) as hpool,
            tc.tile_pool(name="psum", bufs=4, space="PSUM") as ppool,
            tc.tile_pool(name="psumfc", bufs=2, space="PSUM") as fcpool,
        ):
            # ---- constants ----
            w1 = cpool.tile([128, NCH * F], f16)
            w2 = cpool.tile([128, NCH * F], f16)
            b1 = cpool.tile([128, 1], f32)
            b2 = cpool.tile([128, 1], f32)
            wfc1 = cpool.tile([128, 1], f16)
            wfc2t = cpool.tile([128, 1], f16)
            wfc2b = cpool.tile([128, 1], f16)
            bfc = cpool.tile([1, 1], f32)
            y_all = cpool.tile([1, b_loc], f32)

            nc.scalar.dma_start(out=w1[:], in_=w1d[:])
            nc.scalar.dma_start(out=w2[:], in_=w2d[:])
            nc.scalar.dma_start(out=b1[:], in_=b1d[:])
            nc.scalar.dma_start(out=b2[:], in_=b2d[:])
            nc.scalar.dma_start(out=wfc1[:], in_=wfc1d[:])
            nc.scalar.dma_start(out=wfc2t[:], in_=wfc2td[:])
            nc.scalar.dma_start(out=wfc2b[:], in_=wfc2bd[:])
            nc.scalar.dma_start(out=bfc[:], in_=bfcd[:])

            Xs = {}       # macro -> X tile
            xds = {}      # macro -> xdup tile
            Hps = {}      # macro -> Hp tile

            def emit_load(m):
                n0 = m * macro
                X = xpool.tile([128, NCH * macro], f16, tag="X")
                # one flat contiguous 8MB copy (host pre-replicated layout)
                nc.sync.dma_start(out=X[:], in_=xrep[m * 128 : (m + 1) * 128, :])
                xd = xdpool.tile([128, macro], f16, tag="xd")
                nc.scalar.dma_start(out=xd[0:64, :], in_=xt[:, n0 : n0 + macro])
                nc.scalar.dma_start(out=xd[64:128, :], in_=xt[:, n0 : n0 + macro])
                Xs[m], xds[m] = X, xd

            def emit_layer(m, w, bvec, hdup_in, X):
                """One CIN layer: z build + contraction + relu epilogue.
                Returns (ha, hb) = psum halves after matmul (pre-activation)."""
                psa = ppool.tile([128, half], f32, tag="ps")
                psb = ppool.tile([128, half], f32, tag="ps")
                for c0 in range(0, NCH, tt_fuse):
                    z = zpool.tile([128, tt_fuse * macro], f16, tag="z")
                    nc.vector.tensor_tensor(
                        z[:].rearrange("p (f n) -> p f n", n=macro),
                        hdup_in.unsqueeze(1).broadcast_to([128, tt_fuse, macro]),
                        X[:, c0 * macro : (c0 + tt_fuse) * macro]
                        .rearrange("p (f n) -> p f n", n=macro),
                        mult,
                    )
                    for cc in range(tt_fuse):
                        c = c0 + cc
                        wsl = w[:, c * F : (c + 1) * F]
                        zoff = cc * macro
                        nc.tensor.matmul(
                            psa[0:64, :], wsl, z[:, zoff : zoff + half],
                            start=(c == 0), stop=(c == NCH - 1),
                        )
                        nc.tensor.matmul(
                            psb[64:128, :], wsl, z[:, zoff + half : zoff + macro],
                            start=(c == 0), stop=(c == NCH - 1),
                        )
                return psa, psb

            def emit_l1(m):
                psa, psb = emit_layer(m, w1, b1, xds[m][:, :], Xs[m])
                Hp = hpool.tile([128, macro], f16, tag="Hp")
                nc.scalar.activation(
                    Hp[0:64, 0:half], psa[0:64, :], Relu, bias=b1[0:64, :]
                )
                nc.scalar.activation(
                    Hp[64:128, half:macro], psb[64:128, :], Relu, bias=b1[64:128, :]
                )
                # cross-partition duplicates (engines are lane-locked -> DMA)
                nc.scalar.dma_start(out=Hp[64:128, 0:half], in_=Hp[0:64, 0:half])
                nc.scalar.dma_start(
                    out=Hp[0:64, half:macro], in_=Hp[64:128, half:macro]
                )
                Hps[m] = Hp

            def emit_l2(m):
                Hp, X = Hps[m], Xs[m]
                psa, psb = emit_layer(m, w2, b2, Hp[:, :], X)
                h2 = hpool.tile([128, half], f16, tag="h2")
                nc.scalar.activation(
                    h2[0:64, :], psa[0:64, :], Relu, bias=b2[0:64, :]
                )
                nc.scalar.activation(
                    h2[64:128, :], psb[64:128, :], Relu, bias=b2[64:128, :]
                )
                # FC over fields on PE; pfA = n-subtile t, pfB = subtile t+1
                pfA = fcpool.tile([1, half], f32, tag="pf")
                pfB = fcpool.tile([1, half], f32, tag="pf")
                nc.tensor.matmul(
                    pfA[:], wfc1[0:64, :], Hp[0:64, 0:half], start=True, stop=False
                )
                nc.tensor.matmul(pfA[:], wfc2t[:], h2[:, :], start=False, stop=True)
                nc.tensor.matmul(
                    pfB[:], wfc1[0:64, :], Hp[0:64, half:macro],
                    start=True, stop=False,
                )
                nc.tensor.matmul(pfB[:], wfc2b[:], h2[:, :], start=False, stop=True)
                # sum-pool over d
                nbat = half // D
                c0 = m * (macro // D)
                nc.vector.tensor_reduce(
                    y_all[0:1, c0 : c0 + nbat],
                    pfA[0:1, :].rearrange("p (b d) -> p b d", d=D),
                    mybir.AxisListType.X, mybir.AluOpType.add,
                )
                nc.vector.tensor_reduce(
                    y_all[0:1, c0 + nbat : c0 + 2 * nbat],
                    pfB[0:1, :].rearrange("p (b d) -> p b d", d=D),
                    mybir.AxisListType.X, mybir.AluOpType.add,
                )
                del Hps[m], Xs[m], xds[m]

            # ---- depth-2 software pipeline over macro tiles ----
            # X(m+1) DMA is issued before L2(m): the ~20us of L2 DVE work
            # hides the ~23us X transfer, and L1(m+1) follows right after.
            emit_load(0)
            emit_l1(0)
            for m in range(n_macro):
                if m + 1 < n_macro:
                    emit_load(m + 1)
                emit_l2(m)
                if m + 1 < n_macro:
                    emit_l1(m + 1)

            nc.vector.tensor_scalar_add(y_all[:], y_all[:], bfc[:])
            nc.scalar.dma_start(out=yd[:], in_=y_all[:])

    return nc


def _prep_shared(W1, b1, W2, b2, Wfc, bfc):
    """Host-side weight relayout (replicated on every core)."""
    def lay_w(W):
        # w[p, c*F + o] = W[o, c*128 + p]
        wt = np.ascontiguousarray(W.T.astype(np.float16))      # [4096, 64]
        return np.ascontiguousarray(
            wt.reshape(NCH, 128, F).transpose(1, 0, 2).reshape(128, NCH * F)
        )

    return {
        "w1sb": lay_w(W1),
        "w2sb": lay_w(W2),
        "b1dup": np.concatenate([b1, b1]).reshape(128, 1).astype(np.float32),
        "b2dup": np.concatenate([b2, b2]).reshape(128, 1).astype(np.float32),
        "wfc1": np.concatenate([Wfc[0, :F], Wfc[0, :F]]).reshape(128, 1).astype(np.float16),
        "wfc2t": np.concatenate([Wfc[0, F:], np.zeros(F, np.float32)]).reshape(128, 1).astype(np.float16),
        "wfc2b": np.concatenate([np.zeros(F, np.float32), Wfc[0, F:]]).reshape(128, 1).astype(np.float16),
        "bfc": bfc.reshape(1, 1).astype(np.float32),
    }


def _prep_x(xt, b_loc, macro=1024):
    """Build the per-macro-blocked replicated X layout.
    xrep[m*128 + p, c*macro + nn] = xt[2c + p//64, m*macro + nn]"""
    n_total = b_loc * D
    n_macro = n_total // macro
    xm = xt.reshape(F, n_macro, macro)                   # [row, m, nn]
    # rows for (r, c): 2c + r ; partition p = r*64 + j (j broadcast)
    sel = xm.reshape(NCH, 2, n_macro, macro)             # [c, r, m, nn]
    rep = np.broadcast_to(
        sel.transpose(2, 1, 0, 3)[:, :, None, :, :],     # [m, r, 1, c, nn]
        (n_macro, 2, 64, NCH, macro),
    )
    return np.ascontiguousarray(rep).reshape(n_macro * 128, NCH * macro)


_NC_CACHE = {}


def _get_nc(key=(B_LOC, 1024, 2)):
    if key not in _NC_CACHE:
        nc = build_cin_nc(*key)
        nc.finalize()   # bacc legalization (wait splitting, reg alloc)
        _NC_CACHE[key] = nc
    return _NC_CACHE[key]


def run(x, W1, b1, W2, b2, Wfc, bfc, trace=False, macro=1024, tt_fuse=2,
        **spmd_kwargs):
    x = np.asarray(x, dtype=np.float32)
    shared = _prep_shared(
        np.asarray(W1, np.float32), np.asarray(b1, np.float32),
        np.asarray(W2, np.float32), np.asarray(b2, np.float32),
        np.asarray(Wfc, np.float32), np.asarray(bfc, np.float32),
    )
    in_maps = []
    for c in range(NCORES):
        xc = x[c * B_LOC : (c + 1) * B_LOC]                    # [128, F, D]
        xtc = np.ascontiguousarray(
            xc.transpose(1, 0, 2).reshape(F, B_LOC * D).astype(np.float16)
        )
        in_maps.append(
            {"xt16": xtc, "xrep": _prep_x(xtc, B_LOC, macro), **shared}
        )
    nc = _get_nc((B_LOC, macro, tt_fuse))
    res = run_bass_kernel_spmd(
        nc, in_maps, list(range(NCORES)), trace=trace, **spmd_kwargs
    )
    ys = [np.asarray(res.results[i]["y"]).reshape(B_LOC) for i in range(NCORES)]
    out = np.concatenate(ys).reshape(B, 1).astype(np.float32)
    return out, res


def kernel(x, W1, b1, W2, b2, Wfc, bfc):
    out, _ = run(x, W1, b1, W2, b2, Wfc, bfc, trace=False)
    return out


# revision 18
# speedup vs baseline: 1.6251x; 1.1190x over previous
"""CIN (Compressed Interaction Network) forward kernel for Trainium2, 8 cores.

Reference computation (per batch b, per position d):
  h0 = x                                  # [F=64, D=64] fields
  for layer l in (1, 2):
    z[(i,j), d] = x[i, d] * h[j, d]       # outer product, K = F*F = 4096
    h[o, d] = relu(sum_c W_l[o, c] z[c, d] + b_l[o])
  pooled[2F] = sum_d concat(h1, h2)
  y = pooled @ Wfc.T + bfc

Distribution: pure data parallel — batch dim (1024) split 128/core across 8
NeuronCores; weights replicated; no collectives (forward only).

Per-core algorithm ("n" = flattened (batch, d) = 8192 columns, processed in
macro-tiles of 1024 columns):
  - The i-side operand X_c[p, n] = x[2c + p//64, n] (row broadcast across 64
    partitions) is pre-replicated BY THE HOST into a per-macro-blocked DRAM
    tensor, so on-device it's one flat contiguous 8MB DMA per macro at HBM
    line rate (128 descriptors), instead of a scattered broadcast pattern.
  - DVE builds z = Hdup * X in fp16 (2x perf mode), two chunks per op
    (tt_fuse) via a stride-0 middle dim on the h-side AP.
  - PE contracts psum[o, :] += W_chunk^T @ z, the two 512-col n-subtiles
    running concurrently on array column halves (M=64 each, auto
    tile_position) -> full-array rate. Separate PSUM banks per half.
  - ACT applies bias+relu (fp32 psum -> fp16 Hdup); one cross-partition
    duplicate via small SBUF->SBUF DMAs on the scalar HWDGE ring (separate
    from the X ring to avoid head-of-line blocking).
  - Final FC is folded into PE (contract fields, K=128 with zero-padded
    wfc halves), sum-pool over d via DVE tensor_reduce on [1, n] psum.
  - Software pipelined depth 2: L1 of macro m+1 is emitted before L2 of
    macro m, so the DVE never idles across the layer boundary.
"""

import numpy as np

import concourse.bacc as bacc
import concourse.mybir as mybir
import concourse.tile as tile
from concourse.bass_utils import run_bass_kernel_spmd

F = 64          # fields
D = 64          # embedding dim
B = 1024        # full batch
NCORES = 8
B_LOC = B // NCORES          # 128 batches per core
NCH = (F * F) // 128         # 32 contraction chunks of 128
f16 = mybir.dt.float16
f32 = mybir.dt.float32


def build_cin_nc(b_loc=B_LOC, macro=1024, tt_fuse=4):
    n_total = b_loc * D
    assert n_total % macro == 0
    n_macro = n_total // macro
    half = macro // 2
    assert half <= 512  # psum bank limit for fp32
    assert NCH % tt_fuse == 0 and (NCH // 2) % tt_fuse == 0

    nc = bacc.Bacc(None)

    xt = nc.dram_tensor("xt16", [F, n_total], f16, kind="ExternalInput")
    xrep = nc.dram_tensor(
        "xrep", [n_macro * 128, NCH * macro], f16, kind="ExternalInput"
    )
    w1d = nc.dram_tensor("w1sb", [128, NCH * F], f16, kind="ExternalInput")
    w2d = nc.dram_tensor("w2sb", [128, NCH * F], f16, kind="ExternalInput")
    b1d = nc.dram_tensor("b1dup", [128, 1], f32, kind="ExternalInput")
    b2d = nc.dram_tensor("b2dup", [128, 1], f32, kind="ExternalInput")
    wfc1d = nc.dram_tensor("wfc1", [128, 1], f16, kind="ExternalInput")
    wfc2td = nc.dram_tensor("wfc2t", [128, 1], f16, kind="ExternalInput")
    wfc2bd = nc.dram_tensor("wfc2b", [128, 1], f16, kind="ExternalInput")
    bfcd = nc.dram_tensor("bfc", [1, 1], f32, kind="ExternalInput")
    yd = nc.dram_tensor("y", [1, b_loc], f32, kind="ExternalOutput")

    mult = mybir.AluOpType.mult
    Relu = mybir.ActivationFunctionType.Relu

    with tile.TileContext(nc) as tc:
        with (
            tc.tile_pool(name="const", bufs=1) as cpool,
            tc.tile_pool(name="xbig", bufs=3) as xpool,
            tc.tile_pool(name="xd", bufs=3) as xdpool,
            tc.tile_pool(name="z", bufs=4) as zpool,
            tc.tile_pool(name="h", bufs=2) as hpool,
            tc.tile_pool(name="psum", bufs=4, space="PSUM") as ppool,
            tc.tile_pool(name="psumfc", bufs=2, space="PSUM") as fcpool,
        ):
            # ---- constants ----
            w1 = cpool.tile([128, NCH * F], f16)
            w2 = cpool.tile([128, NCH * F], f16)
            b1 = cpool.tile([128, 1], f32)
            b2 = cpool.tile([128, 1], f32)
            wfc1 = cpool.tile([128, 1], f16)
            wfc2t = cpool.tile([128, 1], f16)
            wfc2b = cpool.tile([128, 1], f16)
            bfc = cpool.tile([1, 1], f32)
            y_all = cpool.tile([1, b_loc], f32)

            nc.scalar.dma_start(out=w1[:], in_=w1d[:])
            nc.scalar.dma_start(out=w2[:], in_=w2d[:])
            nc.scalar.dma_start(out=b1[:], in_=b1d[:])
            nc.scalar.dma_start(out=b2[:], in_=b2d[:])
            nc.scalar.dma_start(out=wfc1[:], in_=wfc1d[:])
            nc.scalar.dma_start(out=wfc2t[:], in_=wfc2td[:])
            nc.scalar.dma_start(out=wfc2b[:], in_=wfc2bd[:])
            nc.scalar.dma_start(out=bfc[:], in_=bfcd[:])

            Xs = {}       # macro -> X tile
            xds = {}      # macro -> xdup tile
            Hps = {}      # macro -> Hp tile

            def emit_load(m):
                n0 = m * macro
                hc = NCH // 2
                # two flat contiguous 4MB copies (host pre-replicated layout);
                # half-tiles recycle pool slots at finer grain so the next
                # macro's transfer fully hides behind compute
                Xa = xpool.tile([128, hc * macro], f16, tag="X")
                Xb = xpool.tile([128, hc * macro], f16, tag="X")
                nc.sync.dma_start(
                    out=Xa[:], in_=xrep[m * 128 : (m + 1) * 128, 0 : hc * macro]
                )
                nc.sync.dma_start(
                    out=Xb[:],
                    in_=xrep[m * 128 : (m + 1) * 128, hc * macro : NCH * macro],
                )
                xd = xdpool.tile([128, macro], f16, tag="xd")
                nc.scalar.dma_start(out=xd[0:64, :], in_=xt[:, n0 : n0 + macro])
                nc.scalar.dma_start(out=xd[64:128, :], in_=xt[:, n0 : n0 + macro])
                Xs[m], xds[m] = (Xa, Xb), xd

            def emit_layer(m, w, bvec, hdup_in, X):
                """One CIN layer: z build + contraction + relu epilogue.
                Returns (ha, hb) = psum halves after matmul (pre-activation)."""
                hc = NCH // 2
                psa = ppool.tile([128, half], f32, tag="ps")
                psb = ppool.tile([128, half], f32, tag="ps")
                for c0 in range(0, NCH, tt_fuse):
                    Xh = X[c0 // hc]
                    o0 = (c0 % hc) * macro
                    z = zpool.tile([128, tt_fuse * macro], f16, tag="z")
                    nc.vector.tensor_tensor(
                        z[:].rearrange("p (f n) -> p f n", n=macro),
                        hdup_in.unsqueeze(1).broadcast_to([128, tt_fuse, macro]),
                        Xh[:, o0 : o0 + tt_fuse * macro]
                        .rearrange("p (f n) -> p f n", n=macro),
                        mult,
                    )
                    for cc in range(tt_fuse):
                        c = c0 + cc
                        wsl = w[:, c * F : (c + 1) * F]
                        zoff = cc * macro
                        nc.tensor.matmul(
                            psa[0:64, :], wsl, z[:, zoff : zoff + half],
                            start=(c == 0), stop=(c == NCH - 1),
                        )
                        nc.tensor.matmul(
                            psb[64:128, :], wsl, z[:, zoff + half : zoff + macro],
                            start=(c == 0), stop=(c == NCH - 1),
                        )
                return psa, psb

            def emit_l1(m):
                psa, psb = emit_layer(m, w1, b1, xds[m][:, :], Xs[m])
                Hp = hpool.tile([128, macro], f16, tag="Hp")
                nc.scalar.activation(
                    Hp[0:64, 0:half], psa[0:64, :], Relu, bias=b1[0:64, :]
                )
                nc.scalar.activation(
                    Hp[64:128, half:macro], psb[64:128, :], Relu, bias=b1[64:128, :]
                )
                # cross-partition duplicates (engines are lane-locked -> DMA)
                nc.scalar.dma_start(out=Hp[64:128, 0:half], in_=Hp[0:64, 0:half])
                nc.scalar.dma_start(
                    out=Hp[0:64, half:macro], in_=Hp[64:128, half:macro]
                )
                Hps[m] = Hp

            def emit_l2(m):
                Hp, X = Hps[m], Xs[m]
                psa, psb = emit_layer(m, w2, b2, Hp[:, :], X)
                h2 = hpool.tile([128, half], f16, tag="h2")
                nc.scalar.activation(
                    h2[0:64, :], psa[0:64, :], Relu, bias=b2[0:64, :]
                )
                nc.scalar.activation(
                    h2[64:128, :], psb[64:128, :], Relu, bias=b2[64:128, :]
                )
                # FC over fields on PE; pfA = n-subtile t, pfB = subtile t+1
                pfA = fcpool.tile([1, half], f32, tag="pf")
                pfB = fcpool.tile([1, half], f32, tag="pf")
                nc.tensor.matmul(
                    pfA[:], wfc1[0:64, :], Hp[0:64, 0:half], start=True, stop=False
                )
                nc.tensor.matmul(pfA[:], wfc2t[:], h2[:, :], start=False, stop=True)
                nc.tensor.matmul(
                    pfB[:], wfc1[0:64, :], Hp[0:64, half:macro],
                    start=True, stop=False,
                )
                nc.tensor.matmul(pfB[:], wfc2b[:], h2[:, :], start=False, stop=True)
                # sum-pool over d
                nbat = half // D
                c0 = m * (macro // D)
                nc.vector.tensor_reduce(
                    y_all[0:1, c0 : c0 + nbat],
                    pfA[0:1, :].rearrange("p (b d) -> p b d", d=D),
                    mybir.AxisListType.X, mybir.AluOpType.add,
                )
                nc.vector.tensor_reduce(
                    y_all[0:1, c0 + nbat : c0 + 2 * nbat],
                    pfB[0:1, :].rearrange("p (b d) -> p b d", d=D),
                    mybir.AxisListType.X, mybir.AluOpType.add,
                )
                del Hps[m], Xs[m], xds[m]

            # ---- depth-2 software pipeline over macro tiles ----
            # X(m+1) DMA is issued before L2(m): the ~20us of L2 DVE work
            # hides the ~23us X transfer, and L1(m+1) follows right after.
            emit_load(0)
            emit_l1(0)
            for m in range(n_macro):
                if m + 1 < n_macro:
                    emit_load(m + 1)
                emit_l2(m)
                if m + 1 < n_macro:
                    emit_l1(m + 1)

            nc.vector.tensor_scalar_add(y_all[:], y_all[:], bfc[:])
            nc.scalar.dma_start(out=yd[:], in_=y_all[:])

    return nc


def _prep_shared(W1, b1, W2, b2, Wfc, bfc):
    """Host-side weight relayout (replicated on every core)."""
    def lay_w(W):
        # w[p, c*F + o] = W[o, c*128 + p]
        wt = np.ascontiguousarray(W.T.astype(np.float16))      # [4096, 64]
        return np.ascontiguousarray(
            wt.reshape(NCH, 128, F).transpose(1, 0, 2).reshape(128, NCH * F)
        )

    return {
        "w1sb": lay_w(W1),
        "w2sb": lay_w(W2),
        "b1dup": np.concatenate([b1, b1]).reshape(128, 1).astype(np.float32),
        "b2dup": np.concatenate([b2, b2]).reshape(128, 1).astype(np.float32),
        "wfc1": np.concatenate([Wfc[0, :F], Wfc[0, :F]]).reshape(128, 1).astype(np.float16),
        "wfc2t": np.concatenate([Wfc[0, F:], np.zeros(F, np.float32)]).reshape(128, 1).astype(np.float16),
        "wfc2b": np.concatenate([np.zeros(F, np.float32), Wfc[0, F:]]).reshape(128, 1).astype(np.float16),
        "bfc": bfc.reshape(1, 1).astype(np.float32),
    }


def _prep_x(xt, b_loc, macro=1024):
    """Build the per-macro-blocked replicated X layout.
    xrep[m*128 + p, c*macro + nn] = xt[2c + p//64, m*macro + nn]"""
    n_total = b_loc * D
    n_macro = n_total // macro
    xm = xt.reshape(F, n_macro, macro)                   # [row, m, nn]
    # rows for (r, c): 2c + r ; partition p = r*64 + j (j broadcast)
    sel = xm.reshape(NCH, 2, n_macro, macro)             # [c, r, m, nn]
    rep = np.broadcast_to(
        sel.transpose(2, 1, 0, 3)[:, :, None, :, :],     # [m, r, 1, c, nn]
        (n_macro, 2, 64, NCH, macro),
    )
    return np.ascontiguousarray(rep).reshape(n_macro * 128, NCH * macro)


_NC_CACHE = {}


def _get_nc(key=(B_LOC, 1024, 4)):
    if key not in _NC_CACHE:
        nc = build_cin_nc(*key)
        nc.finalize()   # bacc legalization (wait splitting, reg alloc)
        _NC_CACHE[key] = nc
    return _NC_CACHE[key]


def run(x, W1, b1, W2, b2, Wfc, bfc, trace=False, macro=1024, tt_fuse=4,
        **spmd_kwargs):
    x = np.asarray(x, dtype=np.float32)
    shared = _prep_shared(
        np.asarray(W1, np.float32), np.asarray(b1, np.float32),
        np.asarray(W2, np.float32), np.asarray(b2, np.float32),
        np.asarray(Wfc, np.float32), np.asarray(bfc, np.float32),
    )
    in_maps = []
    for c in range(NCORES):
        xc = x[c * B_LOC : (c + 1) * B_LOC]                    # [128, F, D]
        xtc = np.ascontiguousarray(
            xc.transpose(1, 0, 2).reshape(F, B_LOC * D).astype(np.float16)
        )
        in_maps.append(
            {"xt16": xtc, "xrep": _prep_x(xtc, B_LOC, macro), **shared}
        )
    nc = _get_nc((B_LOC, macro, tt_fuse))
    res = run_bass_kernel_spmd(
        nc, in_maps, list(range(NCORES)), trace=trace, **spmd_kwargs
    )
    ys = [np.asarray(res.results[i]["y"]).reshape(B_LOC) for i in range(NCORES)]
    out = np.concatenate(ys).reshape(B, 1).astype(np.float32)
    return out, res


def kernel(x, W1, b1, W2, b2, Wfc, bfc):
    out, _ = run(x, W1, b1, W2, b2, Wfc, bfc, trace=False)
    return out


# revision 20
# speedup vs baseline: 1.6474x; 1.0137x over previous
"""CIN (Compressed Interaction Network) forward kernel for Trainium2, 8 cores.

Reference computation (per batch b, per position d):
  h0 = x                                  # [F=64, D=64] fields
  for layer l in (1, 2):
    z[(i,j), d] = x[i, d] * h[j, d]       # outer product, K = F*F = 4096
    h[o, d] = relu(sum_c W_l[o, c] z[c, d] + b_l[o])
  pooled[2F] = sum_d concat(h1, h2)
  y = pooled @ Wfc.T + bfc

Distribution: pure data parallel — batch dim (1024) split 128/core across 8
NeuronCores; weights replicated; no collectives (forward only).

Per-core algorithm ("n" = flattened (batch, d) = 8192 columns, processed in
macro-tiles of 1024 columns):
  - The i-side operand X_c[p, n] = x[2c + p//64, n] (row broadcast across 64
    partitions) is pre-replicated BY THE HOST into a per-macro-blocked DRAM
    tensor, so on-device it's one flat contiguous 8MB DMA per macro at HBM
    line rate (128 descriptors), instead of a scattered broadcast pattern.
  - DVE builds z = Hdup * X in fp16 (2x perf mode), two chunks per op
    (tt_fuse) via a stride-0 middle dim on the h-side AP.
  - PE contracts psum[o, :] += W_chunk^T @ z, the two 512-col n-subtiles
    running concurrently on array column halves (M=64 each, auto
    tile_position) -> full-array rate. Separate PSUM banks per half.
  - ACT applies bias+relu (fp32 psum -> fp16 Hdup); one cross-partition
    duplicate via small SBUF->SBUF DMAs on the scalar HWDGE ring (separate
    from the X ring to avoid head-of-line blocking).
  - Final FC is folded into PE (contract fields, K=128 with zero-padded
    wfc halves), sum-pool over d via DVE tensor_reduce on [1, n] psum.
  - Software pipelined depth 2: L1 of macro m+1 is emitted before L2 of
    macro m, so the DVE never idles across the layer boundary.
"""

import numpy as np

import concourse.bacc as bacc
import concourse.mybir as mybir
import concourse.tile as tile
from concourse.bass_utils import run_bass_kernel_spmd

F = 64          # fields
D = 64          # embedding dim
B = 1024        # full batch
NCORES = 8
B_LOC = B // NCORES          # 128 batches per core
NCH = (F * F) // 128         # 32 contraction chunks of 128
f16 = mybir.dt.float16
f32 = mybir.dt.float32


def build_cin_nc(b_loc=B_LOC, macro=1024, tt_fuse=4):
    n_total = b_loc * D
    assert n_total % macro == 0
    n_macro = n_total // macro
    half = macro // 2
    assert half <= 512  # psum bank limit for fp32
    assert NCH % tt_fuse == 0 and (NCH // 2) % tt_fuse == 0

    nc = bacc.Bacc(None)

    xt = nc.dram_tensor("xt16", [F, n_total], f16, kind="ExternalInput")
    xrep = nc.dram_tensor(
        "xrep", [n_macro * 128, NCH * macro], f16, kind="ExternalInput"
    )
    w1d = nc.dram_tensor("w1sb", [128, NCH * F], f16, kind="ExternalInput")
    w2d = nc.dram_tensor("w2sb", [128, NCH * F], f16, kind="ExternalInput")
    b1d = nc.dram_tensor("b1dup", [128, 1], f32, kind="ExternalInput")
    b2d = nc.dram_tensor("b2dup", [128, 1], f32, kind="ExternalInput")
    wfc1d = nc.dram_tensor("wfc1", [128, 1], f16, kind="ExternalInput")
    wfc2td = nc.dram_tensor("wfc2t", [128, 1], f16, kind="ExternalInput")
    wfc2bd = nc.dram_tensor("wfc2b", [128, 1], f16, kind="ExternalInput")
    bfcd = nc.dram_tensor("bfc", [1, 1], f32, kind="ExternalInput")
    yd = nc.dram_tensor("y", [1, b_loc], f32, kind="ExternalOutput")

    mult = mybir.AluOpType.mult
    Relu = mybir.ActivationFunctionType.Relu

    with tile.TileContext(nc) as tc:
        with (
            tc.tile_pool(name="const", bufs=1) as cpool,
            tc.tile_pool(name="xbig", bufs=3) as xpool,
            tc.tile_pool(name="xd", bufs=3) as xdpool,
            tc.tile_pool(name="z", bufs=6) as zpool,
            tc.tile_pool(name="h", bufs=2) as hpool,
            tc.tile_pool(name="psum", bufs=4, space="PSUM") as ppool,
            tc.tile_pool(name="psumfc", bufs=2, space="PSUM") as fcpool,
        ):
            # ---- constants ----  (X(0) DMA is issued first, see below)
            w1 = cpool.tile([128, NCH * F], f16)
            w2 = cpool.tile([128, NCH * F], f16)
            b1 = cpool.tile([128, 1], f32)
            b2 = cpool.tile([128, 1], f32)
            wfc1 = cpool.tile([128, 1], f16)
            wfc2t = cpool.tile([128, 1], f16)
            wfc2b = cpool.tile([128, 1], f16)
            bfc = cpool.tile([1, 1], f32)
            y_all = cpool.tile([1, b_loc], f32)

            Xs = {}       # macro -> X tile
            xds = {}      # macro -> xdup tile
            Hps = {}      # macro -> Hp tile

            def emit_load(m):
                n0 = m * macro
                hc = NCH // 2
                # two flat contiguous 4MB copies (host pre-replicated layout);
                # half-tiles recycle pool slots at finer grain so the next
                # macro's transfer fully hides behind compute
                Xa = xpool.tile([128, hc * macro], f16, tag="X")
                Xb = xpool.tile([128, hc * macro], f16, tag="X")
                nc.sync.dma_start(
                    out=Xa[:], in_=xrep[m * 128 : (m + 1) * 128, 0 : hc * macro]
                )
                nc.sync.dma_start(
                    out=Xb[:],
                    in_=xrep[m * 128 : (m + 1) * 128, hc * macro : NCH * macro],
                )
                xd = xdpool.tile([128, macro], f16, tag="xd")
                nc.scalar.dma_start(out=xd[0:64, :], in_=xt[:, n0 : n0 + macro])
                nc.scalar.dma_start(out=xd[64:128, :], in_=xt[:, n0 : n0 + macro])
                Xs[m], xds[m] = (Xa, Xb), xd

            def emit_layer(m, w, bvec, hdup_in, X):
                """One CIN layer: z build + contraction + relu epilogue.
                Returns (ha, hb) = psum halves after matmul (pre-activation)."""
                hc = NCH // 2
                psa = ppool.tile([128, half], f32, tag="ps")
                psb = ppool.tile([128, half], f32, tag="ps")
                for c0 in range(0, NCH, tt_fuse):
                    Xh = X[c0 // hc]
                    o0 = (c0 % hc) * macro
                    z = zpool.tile([128, tt_fuse * macro], f16, tag="z")
                    nc.vector.tensor_tensor(
                        z[:].rearrange("p (f n) -> p f n", n=macro),
                        hdup_in.unsqueeze(1).broadcast_to([128, tt_fuse, macro]),
                        Xh[:, o0 : o0 + tt_fuse * macro]
                        .rearrange("p (f n) -> p f n", n=macro),
                        mult,
                    )
                    for cc in range(tt_fuse):
                        c = c0 + cc
                        wsl = w[:, c * F : (c + 1) * F]
                        zoff = cc * macro
                        nc.tensor.matmul(
                            psa[0:64, :], wsl, z[:, zoff : zoff + half],
                            start=(c == 0), stop=(c == NCH - 1),
                        )
                        nc.tensor.matmul(
                            psb[64:128, :], wsl, z[:, zoff + half : zoff + macro],
                            start=(c == 0), stop=(c == NCH - 1),
                        )
                return psa, psb

            def emit_l1(m):
                psa, psb = emit_layer(m, w1, b1, xds[m][:, :], Xs[m])
                Hp = hpool.tile([128, macro], f16, tag="Hp")
                nc.scalar.activation(
                    Hp[0:64, 0:half], psa[0:64, :], Relu, bias=b1[0:64, :]
                )
                nc.scalar.activation(
                    Hp[64:128, half:macro], psb[64:128, :], Relu, bias=b1[64:128, :]
                )
                # cross-partition duplicates (engines are lane-locked -> DMA)
                nc.scalar.dma_start(out=Hp[64:128, 0:half], in_=Hp[0:64, 0:half])
                nc.scalar.dma_start(
                    out=Hp[0:64, half:macro], in_=Hp[64:128, half:macro]
                )
                Hps[m] = Hp

            def emit_l2(m):
                Hp, X = Hps[m], Xs[m]
                psa, psb = emit_layer(m, w2, b2, Hp[:, :], X)
                h2 = hpool.tile([128, half], f16, tag="h2")
                nc.scalar.activation(
                    h2[0:64, :], psa[0:64, :], Relu, bias=b2[0:64, :]
                )
                nc.scalar.activation(
                    h2[64:128, :], psb[64:128, :], Relu, bias=b2[64:128, :]
                )
                # FC over fields on PE; pfA = n-subtile t, pfB = subtile t+1
                pfA = fcpool.tile([1, half], f32, tag="pf")
                pfB = fcpool.tile([1, half], f32, tag="pf")
                nc.tensor.matmul(
                    pfA[:], wfc1[0:64, :], Hp[0:64, 0:half], start=True, stop=False
                )
                nc.tensor.matmul(pfA[:], wfc2t[:], h2[:, :], start=False, stop=True)
                nc.tensor.matmul(
                    pfB[:], wfc1[0:64, :], Hp[0:64, half:macro],
                    start=True, stop=False,
                )
                nc.tensor.matmul(pfB[:], wfc2b[:], h2[:, :], start=False, stop=True)
                # sum-pool over d
                nbat = half // D
                c0 = m * (macro // D)
                nc.vector.tensor_reduce(
                    y_all[0:1, c0 : c0 + nbat],
                    pfA[0:1, :].rearrange("p (b d) -> p b d", d=D),
                    mybir.AxisListType.X, mybir.AluOpType.add,
                )
                nc.vector.tensor_reduce(
                    y_all[0:1, c0 + nbat : c0 + 2 * nbat],
                    pfB[0:1, :].rearrange("p (b d) -> p b d", d=D),
                    mybir.AxisListType.X, mybir.AluOpType.add,
                )
                del Hps[m], Xs[m], xds[m]

            # ---- depth-2 software pipeline over macro tiles ----
            # X(0) first so its transfer overlaps the const doorbells
            emit_load(0)
            nc.scalar.dma_start(out=w1[:], in_=w1d[:])
            nc.scalar.dma_start(out=w2[:], in_=w2d[:])
            nc.scalar.dma_start(out=b1[:], in_=b1d[:])
            nc.scalar.dma_start(out=b2[:], in_=b2d[:])
            nc.scalar.dma_start(out=wfc1[:], in_=wfc1d[:])
            nc.scalar.dma_start(out=wfc2t[:], in_=wfc2td[:])
            nc.scalar.dma_start(out=wfc2b[:], in_=wfc2bd[:])
            nc.scalar.dma_start(out=bfc[:], in_=bfcd[:])
            emit_l1(0)
            for m in range(n_macro):
                if m + 1 < n_macro:
                    emit_load(m + 1)
                emit_l2(m)
                if m + 1 < n_macro:
                    emit_l1(m + 1)

            nc.vector.tensor_scalar_add(y_all[:], y_all[:], bfc[:])
            nc.scalar.dma_start(out=yd[:], in_=y_all[:])

    return nc


def _prep_shared(W1, b1, W2, b2, Wfc, bfc):
    """Host-side weight relayout (replicated on every core)."""
    def lay_w(W):
        # w[p, c*F + o] = W[o, c*128 + p]
        wt = np.ascontiguousarray(W.T.astype(np.float16))      # [4096, 64]
        return np.ascontiguousarray(
            wt.reshape(NCH, 128, F).transpose(1, 0, 2).reshape(128, NCH * F)
        )

    return {
        "w1sb": lay_w(W1),
        "w2sb": lay_w(W2),
        "b1dup": np.concatenate([b1, b1]).reshape(128, 1).astype(np.float32),
        "b2dup": np.concatenate([b2, b2]).reshape(128, 1).astype(np.float32),
        "wfc1": np.concatenate([Wfc[0, :F], Wfc[0, :F]]).reshape(128, 1).astype(np.float16),
        "wfc2t": np.concatenate([Wfc[0, F:], np.zeros(F, np.float32)]).reshape(128, 1).astype(np.float16),
        "wfc2b": np.concatenate([np.zeros(F, np.float32), Wfc[0, F:]]).reshape(128, 1).astype(np.float16),
        "bfc": bfc.reshape(1, 1).astype(np.float32),
    }


def _prep_x(xt, b_loc, macro=1024):
    """Build the per-macro-blocked replicated X layout.
    xrep[m*128 + p, c*macro + nn] = xt[2c + p//64, m*macro + nn]"""
    n_total = b_loc * D
    n_macro = n_total // macro
    xm = xt.reshape(F, n_macro, macro)                   # [row, m, nn]
    # rows for (r, c): 2c + r ; partition p = r*64 + j (j broadcast)
    sel = xm.reshape(NCH, 2, n_macro, macro)             # [c, r, m, nn]
    rep = np.broadcast_to(
        sel.transpose(2, 1, 0, 3)[:, :, None, :, :],     # [m, r, 1, c, nn]
        (n_macro, 2, 64, NCH, macro),
    )
    return np.ascontiguousarray(rep).reshape(n_macro * 128, NCH * macro)


_NC_CACHE = {}


def _get_nc(key=(B_LOC, 1024, 4)):
    if key not in _NC_CACHE:
        nc = build_cin_nc(*key)
        nc.finalize()   # bacc legalization (wait splitting, reg alloc)
        _NC_CACHE[key] = nc
    return _NC_CACHE[key]


def run(x, W1, b1, W2, b2, Wfc, bfc, trace=False, macro=1024, tt_fuse=4,
        **spmd_kwargs):
    x = np.asarray(x, dtype=np.float32)
    shared = _prep_shared(
        np.asarray(W1, np.float32), np.asarray(b1, np.float32),
        np.asarray(W2, np.float32), np.asarray(b2, np.float32),
        np.asarray(Wfc, np.float32), np.asarray(bfc, np.float32),
    )
    in_maps = []
    for c in range(NCORES):
        xc = x[c * B_LOC : (c + 1) * B_LOC]                    # [128, F, D]
        xtc = np.ascontiguousarray(
            xc.transpose(1, 0, 2).reshape(F, B_LOC * D).astype(np.float16)
        )
        in_maps.append(
            {"xt16": xtc, "xrep": _prep_x(xtc, B_LOC, macro), **shared}
        )
    nc = _get_nc((B_LOC, macro, tt_fuse))
    res = run_bass_kernel_spmd(
        nc, in_maps, list(range(NCORES)), trace=trace, **spmd_kwargs
    )
    ys = [np.asarray(res.results[i]["y"]).reshape(B_LOC) for i in range(NCORES)]
    out = np.concatenate(ys).reshape(B, 1).astype(np.float32)
    return out, res


def kernel(x, W1, b1, W2, b2, Wfc, bfc):
    out, _ = run(x, W1, b1, W2, b2, Wfc, bfc, trace=False)
    return out
